# revision 31
# baseline (speedup 1.0000x reference)
"""Trainium2 Bass kernel for a 12-head causal attention block (B=4, T=2048, C=768).

Sharding: 8 cores = 4 batches x 2 head-groups (6 heads each). Each core computes
q/k/v projections for its head-group over its batch's full sequence, causal
flash-style attention, and a partial output projection (row-parallel Wp).
Host sums the two partial outputs per batch. No cross-core collectives.

All matmul operands are fp16 (fp32 PSUM accumulation). Layouts are channel-major
so no on-chip transposes are needed:
  xT   [768, 2048]  x[b].T                        (fp16)
  wq/wk/wv [768, 384]  W[g*384:(g+1)*384, :].T    (fp16, lhsT layout)
  wp   [384, 768]  Wp[:, g*384:(g+1)*384].T       (fp16, lhsT layout)
  tri  [128, 128] causal triangle tile            (fp16)
  out yT [768, 2048] fp16 partial = (attn_out_group @ Wp_group.T).T

Softmax denominator is folded into the PV matmul: v tiles carry a 65th
column of ones, so each per-head PSUM accumulator [65, 512] holds the
unnormalized attention output in rows 0:64 and the denominator in row 64.
Diagonal s-tiles only compute/accumulate the non-masked column range
(triangular blocking). Normalization runs off the PE critical path:
DVE reciprocal of the denominator row, stride-0-source DMA broadcast
across partitions, GPSIMD multiply into the attention-out tile.

Engine budget per core (approx): PE ~150us, ACT ~120us (exp + q/k PSUM
copies), DVE ~60us (v/y/accumulator copies + recips), GPSIMD ~60us
(causal-mask + normalize multiplies).
"""

import numpy as np

T = 2048
C = 768
G = 384          # channels per head-group (6 heads x 64)
DH = 64
NK = C // 128    # 6 k-tiles over c_in
TBLK = 512
NTB = T // TBLK  # 4 t-blocks
NST = T // 128   # 16 s-tiles
N_CORES = 8

_CACHE = {}


def _emit(tc, yT, xT, wq, wk, wv, wp, tri, dbg=None):
    import concourse.mybir as mybir
    import concourse.bass as bass

    nc = tc.nc
    # DRAM bounce buffer for the denominator-reciprocal broadcast: SBUF
    # sources cannot have a 0-step partition dim in DMA APs, DRAM can.
    # One slot per (t-block, head-pair): no reuse, no WAR.
    nrm_d = nc.dram_tensor("nrm_scratch", [NTB * 3, 2, TBLK],
                           mybir.dt.float16, kind="Internal").ap()
    DT = mybir.dt.float32
    H = mybir.dt.float16
    Exp = mybir.ActivationFunctionType.Exp
    mm = nc.tensor.matmul

    with (
        tc.tile_pool(name="pc", bufs=1) as pc,        # persistent sbuf
        tc.tile_pool(name="px", bufs=2) as px,        # x chunks
        tc.tile_pool(name="pe", bufs=6) as pe,        # exp tiles
        tc.tile_pool(name="pr", bufs=3) as pr,        # y-out staging
        tc.tile_pool(name="pn", bufs=2) as pn,        # norm staging
        tc.tile_pool(name="pao", bufs=2) as pao,      # attn-out per t-block
        tc.tile_pool(name="psP", bufs=2, space="PSUM") as psP,    # scores
        tc.tile_pool(name="psQ", bufs=2, space="PSUM") as psQ,    # projections
        tc.tile_pool(name="psA", bufs=1, space="PSUM") as psA,    # attn accum head even
        tc.tile_pool(name="psB", bufs=1, space="PSUM") as psB,    # attn accum head odd
    ):
        # ---- persistent tensors ----
        wq_sb = pc.tile([128, NK * G], H, tag="wq")
        wk_sb = pc.tile([128, NK * G], H, tag="wk")
        wv_sb = pc.tile([128, NK * G], H, tag="wv")
        wp_sb = pc.tile([128, 3 * C], H, tag="wp")
        tri_sb = pc.tile([128, 128], H, tag="tri")

        # qT/kT: [128, 3*2048]; channel c of group -> partition c%128, block c//128.
        # head h (0..5): partitions (h%2)*64..+64 of block h//2.
        qT_sb = pc.tile([128, 3 * T], H, tag="qT")
        kT_sb = pc.tile([128, 3 * T], H, tag="kT")
        # v token-major with ones column: [128, 16, 6, 65];
        # v65[p, st, h, m] = v[st*128 + p, h*64 + m] for m < 64; col 64 = 1.0
        v65_sb = pc.tile([128, NST, 6, 65], H, tag="v65")

        xT_r = xT.rearrange("(k p) t -> p k t", p=128)
        wq_r = wq.rearrange("(k p) c -> p k c", p=128)
        wk_r = wk.rearrange("(k p) c -> p k c", p=128)
        wv_r = wv.rearrange("(k p) c -> p k c", p=128)
        wq_v = wq_sb.rearrange("p (k c) -> p k c", k=NK)
        wk_v = wk_sb.rearrange("p (k c) -> p k c", k=NK)
        wv_v = wv_sb.rearrange("p (k c) -> p k c", k=NK)

        def load_chunks(tb, split=False):
            xt = px.tile([128, NK, TBLK], H, tag="xc")
            if split:
                for k in range(NK):
                    nc.sync.dma_start(out=xt[:, k, :],
                                      in_=xT_r[:, k, tb * TBLK:(tb + 1) * TBLK])
            else:
                nc.sync.dma_start(out=xt[:], in_=xT_r[:, :, tb * TBLK:(tb + 1) * TBLK])
            return [xt[:, k, :] for k in range(NK)]

        # startup DMAs in priority order: (wq,x0) for the first q projection,
        # then wv (v groups interleave into tb=0 attention early), then wk,
        # then the late weights.
        for k in range(NK):
            nc.sync.dma_start(out=wq_v[:, k, :], in_=wq_r[:, k, :])
        xc_cur = load_chunks(0, split=True)
        for k in range(NK):
            nc.sync.dma_start(out=wk_v[:, k, :], in_=wk_r[:, k, :])
        for k in range(NK):
            nc.sync.dma_start(out=wv_v[:, k, :], in_=wv_r[:, k, :])
        nc.sync.dma_start(out=tri_sb[:], in_=tri)
        nc.sync.dma_start(out=wp_sb.rearrange("p (k c) -> p k c", k=3),
                          in_=wp.rearrange("(k p) c -> p k c", p=128))
        nc.gpsimd.memset(v65_sb[:, :, :, 64:65], 1.0)

        def ph1_groups(tb, xc):
            # closures: one projection matmul group each (q/k x 3, v x 4).
            # q/k PSUM->SBUF copies ride the ACT engine (DVE is the scarcer
            # resource); v copies stay on DVE (strided output).
            gs = []
            for w_sb, out_sb in ((wq_sb, qT_sb), (wk_sb, kT_sb)):
                for mo in range(3):
                    def g(w_sb=w_sb, out_sb=out_sb, mo=mo):
                        ps = psQ.tile([128, TBLK], DT, tag="pq")
                        for k in range(NK):
                            mm(ps[:, 0:TBLK],
                               lhsT=w_sb[:, k * G + mo * 128: k * G + (mo + 1) * 128],
                               rhs=xc[k], start=(k == 0), stop=(k == NK - 1))
                        nc.scalar.copy(
                            out=out_sb[:, mo * T + tb * TBLK: mo * T + (tb + 1) * TBLK],
                            in_=ps[:, 0:TBLK])
                    gs.append(g)
            for sl in range(4):
                def g(sl=sl):
                    st = 4 * tb + sl
                    ps = psQ.tile([128, TBLK], DT, tag="pq")
                    for k in range(NK):
                        mm(ps[:, 0:G], lhsT=xc[k][:, sl * 128:(sl + 1) * 128],
                           rhs=wv_sb[:, k * G:(k + 1) * G], start=(k == 0), stop=(k == NK - 1))
                    nc.vector.tensor_copy(
                        out=v65_sb[:, st, :, 0:64],
                        in_=ps[:, 0:G].rearrange("p (h m) -> p h m", h=6))
                gs.append(g)
            return gs

        def ph3_groups(tb, ao):
            gs = []
            for mo in range(6):
                def g(mo=mo):
                    py = psQ.tile([128, TBLK], DT, tag="pq")
                    for kk in range(3):
                        mm(py[:, 0:TBLK],
                           lhsT=wp_sb[:, kk * C + mo * 128: kk * C + (mo + 1) * 128],
                           rhs=ao[:, kk * TBLK:(kk + 1) * TBLK], start=(kk == 0), stop=(kk == 2))
                    yo = pr.tile([128, TBLK], H, tag="yo")
                    nc.vector.tensor_copy(out=yo[:], in_=py[:, 0:TBLK])
                    nc.sync.dma_start(
                        out=yT[mo * 128:(mo + 1) * 128, tb * TBLK:(tb + 1) * TBLK], in_=yo[:])
                gs.append(g)
            return gs

        # minimal serial head: just the mo=0 q/k projections; everything else
        # (v groups, mo=1/2 q/k) interleaves into tb=0's attention, ordered so
        # each group lands just before its first consumer.
        all0 = ph1_groups(0, xc_cur)    # [q0 q1 q2 k0 k1 k2 v0 v1 v2 v3]
        all0[0]()
        all0[3]()
        tb0_extra = [all0[1], all0[4], all0[6], all0[7],
                     all0[8], all0[2], all0[9], all0[5]]

        queue = []       # projection groups to interleave into phase 2
        ph3_pending = []
        for tb in range(NTB):
            if tb < NTB - 1:
                # ph1 first: those groups are dependency-free, so the PE queue
                # never head-of-line blocks on the previous block's norm chain
                xc_next = load_chunks(tb + 1)
                queue = ph1_groups(tb + 1, xc_next) + ph3_pending
            else:
                queue = list(ph3_pending)
            if tb == 0:
                queue = tb0_extra + queue
            total_iters = 3 * 4 * (tb + 1)
            emitted = [0]

            def make_pop(queue, emitted, total_iters):
                def pop_queue(it):
                    want = min(len(queue), (it + 1) * len(queue) // total_iters + 1)
                    while emitted[0] < want:
                        queue[emitted[0]]()
                        emitted[0] += 1
                return pop_queue
            pop_queue = make_pop(queue, emitted, total_iters)

            # ---- phase 2: attention for this t-block, head pairs j ----
            ao = pao.tile([128, 3 * TBLK], H, tag="ao")
            n_st = 4 * (tb + 1)

            norm_q = []

            def queue_norm(sA, sB, j_, tb_=tb):
                # softmax normalization, entirely off the PE path. The PSUM
                # accumulators were already freed by the per-region copies in
                # pv_group (sA/sB hold unnorm out rows 0:64 + den row 64).
                # recip (approx, DVE) -> DRAM bounce -> stride-0 DMA partition
                # broadcast -> GPSIMD multiplies into ao.
                sBp = pn.tile([128, TBLK], DT, tag="sBp")
                rT = pn.tile([65, 2, TBLK], H, tag="rT")
                dd = pn.tile([128, TBLK], H, tag="dd")
                slot = tb_ * 3 + j_

                nc.gpsimd.dma_start(out=sBp[64:128, :], in_=sB[0:64, :])

                # plain DVE reciprocal (3.3us each, but reads SBUF staging so
                # it never blocks the PE; ln/exp-on-ACT alternates activation
                # tables at 1.3us per switch and starves the exps)
                def g4():
                    with nc.allow_low_precision(reason="1/den in fp16: 5e-4 rel"):
                        nc.vector.reciprocal(out=rT[64:65, 0, :], in_=sA[64:65, :])

                def g5():
                    with nc.allow_low_precision(reason="1/den in fp16: 5e-4 rel"):
                        nc.vector.reciprocal(out=rT[64:65, 1, :], in_=sB[64:65, :])

                def g7():
                    nc.gpsimd.dma_start(out=nrm_d[slot], in_=rT[64:65, :, :])

                def g8():
                    for h, pp in ((0, slice(0, 64)), (1, slice(64, 128))):
                        row = nrm_d[slot, h, :]
                        bc = bass.AP(tensor=row.tensor, offset=row.offset,
                                     ap=[[0, 64], [1, TBLK]])
                        nc.gpsimd.dma_start(out=dd[pp, :], in_=bc)

                def g9():
                    nc.gpsimd.tensor_mul(
                        ao[0:64, j_ * TBLK:(j_ + 1) * TBLK], sA[0:64, :], dd[0:64, :])

                def g10():
                    nc.gpsimd.tensor_mul(
                        ao[64:128, j_ * TBLK:(j_ + 1) * TBLK], sBp[64:128, :],
                        dd[64:128, :])

                norm_q.extend([g4, g5, g7, g8, g9, g10])

            def pv_group(ent):
                # triangular PV: diagonal s-tiles only touch cols >= lo.
                # column region [128p, 128p+128) gets its final accumulation
                # at diagonal tile o == p -> stop=True there, and is copied
                # out to SBUF immediately (region-sized copies decouple the
                # next head-pair's PV start from one big end-of-j copy).
                j_, poA_, poB_, sA_, sB_, st, e01, first, lo = ent
                for h, po_, s_ in ((0, poA_, sA_), (1, poB_, sB_)):
                    v65 = v65_sb[:, st, 2 * j_ + h, :]
                    eh = e01[:, h * TBLK:(h + 1) * TBLK]
                    if lo < 0:       # off-diagonal: full width, never last
                        mm(po_[0:65, 0:TBLK], lhsT=v65, rhs=eh,
                           start=first, stop=False, skip_group_check=True)
                    else:
                        hi = lo + 128
                        mm(po_[0:65, lo:hi], lhsT=v65, rhs=eh[:, lo:hi],
                           start=first, stop=True, skip_group_check=True)
                        if hi < TBLK:
                            mm(po_[0:65, hi:TBLK], lhsT=v65, rhs=eh[:, hi:TBLK],
                               start=first, stop=False, skip_group_check=True)
                        nc.vector.tensor_copy(out=s_[0:65, lo:hi],
                                              in_=po_[0:65, lo:hi])

            pipe = []  # software pipeline: PV(st-2) is issued after scores(st)
            for j in range(3):
                poA = psA.tile([65, TBLK], DT, tag="poA")
                poB = psB.tile([65, TBLK], DT, tag="poB")
                sA = pn.tile([65, TBLK], DT, tag="sA")
                sB = pn.tile([65, TBLK], DT, tag="sB")
                qs = qT_sb[:, j * T + tb * TBLK: j * T + (tb + 1) * TBLK]
                for st in range(n_st):
                    o = st - 4 * tb           # >= 0 on diagonal tiles
                    lo = 128 * o if o >= 0 else -1
                    ks = kT_sb[:, j * T + st * 128: j * T + st * 128 + 128]
                    ps = psP.tile([128, 2 * TBLK], DT, tag="pp")
                    e01 = pe.tile([128, 2 * TBLK], H, tag="e01")
                    if o > 0:
                        # diagonal block: cols < lo are fully masked; compute
                        # scores/exp only on the remainder
                        mm(ps[:, lo:TBLK], lhsT=ks[0:64, :], rhs=qs[0:64, lo:TBLK],
                           start=True, stop=True)
                        mm(ps[:, TBLK + lo:2 * TBLK], lhsT=ks[64:128, :],
                           rhs=qs[64:128, lo:TBLK], start=True, stop=True)
                        nc.scalar.activation(
                            out=e01.rearrange("p (a b) -> p a b", a=2)[:, :, lo:],
                            in_=ps.rearrange("p (a b) -> p a b", a=2)[:, :, lo:],
                            func=Exp, scale=float(DH) ** -0.5)
                    else:
                        mm(ps[:, 0:TBLK], lhsT=ks[0:64, :], rhs=qs[0:64, :],
                           start=True, stop=True)
                        mm(ps[:, TBLK:2 * TBLK], lhsT=ks[64:128, :], rhs=qs[64:128, :],
                           start=True, stop=True)
                        nc.scalar.activation(out=e01[:], in_=ps[:], func=Exp,
                                             scale=float(DH) ** -0.5)
                    if o >= 0:  # triangular boundary block: causal mask (GPSIMD)
                        hi = lo + 128
                        nc.gpsimd.tensor_mul(e01[:, lo:hi], e01[:, lo:hi], tri_sb[:])
                        nc.gpsimd.tensor_mul(e01[:, TBLK + lo:TBLK + hi],
                                             e01[:, TBLK + lo:TBLK + hi], tri_sb[:])
                    pipe.append((j, poA, poB, sA, sB, st, e01, st == 0, lo))
                    if len(pipe) > 2:
                        pv_group(pipe.pop(0))
                    if norm_q:
                        norm_q.pop(0)()  # previous j's normalize, off the critical path
                    pop_queue(j * n_st + st)
                for ent in pipe:   # flush within j (poA/poB are single-buffered)
                    pv_group(ent)
                pipe = []
                queue_norm(sA, sB, j)
            while norm_q:
                norm_q.pop(0)()
            while emitted[0] < len(queue):
                queue[emitted[0]]()
                emitted[0] += 1
            ph3_pending = ph3_groups(tb, ao)
        for g in ph3_pending:  # tail: projection of the last t-block
            g()


def build_program():
    if "nc" in _CACHE:
        return _CACHE["nc"]
    import concourse.bacc as bacc
    import concourse.tile as tile
    import concourse.mybir as mybir

    nc = bacc.Bacc("TRN2", target_bir_lowering=False, debug=False)
    H = mybir.dt.float16
    xT_d = nc.dram_tensor("xT", [C, T], H, kind="ExternalInput")
    wq_d = nc.dram_tensor("wq", [C, G], H, kind="ExternalInput")
    wk_d = nc.dram_tensor("wk", [C, G], H, kind="ExternalInput")
    wv_d = nc.dram_tensor("wv", [C, G], H, kind="ExternalInput")
    wp_d = nc.dram_tensor("wp", [G, C], H, kind="ExternalInput")
    tri_d = nc.dram_tensor("tri", [128, 128], H, kind="ExternalInput")
    yT_d = nc.dram_tensor("yT", [C, T], H, kind="ExternalOutput")

    with tile.TileContext(nc) as tc:
        _emit(tc, yT_d.ap(), xT_d.ap(), wq_d.ap(), wk_d.ap(), wv_d.ap(),
              wp_d.ap(), tri_d.ap())
    nc.compile()
    _CACHE["nc"] = nc
    return nc


def make_tri():
    s = np.arange(128)[:, None]
    t = np.arange(128)[None, :]
    return (t >= s).astype(np.float16)


def shard_inputs(x, Wq, Wk, Wv, Wp):
    """Full inputs -> list of 8 per-core input dicts (fp16 operands)."""
    x = np.asarray(x, dtype=np.float32)
    Wq, Wk, Wv, Wp = (np.asarray(w, dtype=np.float32) for w in (Wq, Wk, Wv, Wp))
    tri = make_tri()
    in_maps = []
    for c in range(N_CORES):
        b, g = divmod(c, 2)
        sl = slice(g * G, (g + 1) * G)
        in_maps.append({
            "xT": np.ascontiguousarray(x[b].T).astype(np.float16),
            "wq": np.ascontiguousarray(Wq[sl, :].T).astype(np.float16),
            "wk": np.ascontiguousarray(Wk[sl, :].T).astype(np.float16),
            "wv": np.ascontiguousarray(Wv[sl, :].T).astype(np.float16),
            "wp": np.ascontiguousarray(Wp[:, sl].T).astype(np.float16),
            "tri": tri,
        })
    return in_maps


def combine_outputs(results):
    """Per-core {'yT': [768,2048]} partials -> full [4, 2048, 768] output."""
    out = np.empty((4, T, C), dtype=np.float32)
    for b in range(4):
        acc = (results[2 * b]["yT"].astype(np.float32)
               + results[2 * b + 1]["yT"].astype(np.float32))
        out[b] = acc.T
    return out


def kernel(x, Wq, Wk, Wv, Wp, **run_kwargs):
    from concourse.bass_utils import run_bass_kernel_spmd

    nc = build_program()
    in_maps = shard_inputs(x, Wq, Wk, Wv, Wp)
    res = run_bass_kernel_spmd(nc, in_maps, core_ids=list(range(N_CORES)), **run_kwargs)
    out = combine_outputs(res.results)
    if run_kwargs:
        return out, res
    return out


# revision 36
# speedup vs baseline: 1.1469x; 1.1469x over previous
"""Trainium2 Bass kernel for a 12-head causal attention block (B=4, T=2048, C=768).

Sharding: 8 cores = 4 batches x 2 head-groups (6 heads each). Each core computes
q/k/v projections for its head-group over its batch's full sequence, causal
flash-style attention, and a partial output projection (row-parallel Wp).
Host sums the two partial outputs per batch. No cross-core collectives.

All matmul operands are fp16 (fp32 PSUM accumulation). Layouts are channel-major
so no on-chip transposes are needed:
  xT   [768, 2048]  x[b].T                        (fp16)
  wq/wk/wv [768, 384]  W[g*384:(g+1)*384, :].T    (fp16, lhsT layout)
  wp   [384, 768]  Wp[:, g*384:(g+1)*384].T       (fp16, lhsT layout)
  tri  [128, 128] causal triangle tile            (fp16)
  out yT [768, 2048] fp16 partial = (attn_out_group @ Wp_group.T).T

Softmax denominator is folded into the PV matmul: v tiles carry a 65th
column of ones, so each per-head PSUM accumulator [65, 512] holds the
unnormalized attention output in rows 0:64 and the denominator in row 64.
Diagonal s-tiles only compute/accumulate the non-masked column range
(triangular blocking). Normalization runs off the PE critical path:
1/den = exp(-ln(den)) on the ACT engine, stride-0-source DMA broadcast
across partitions via a DRAM bounce, GPSIMD multiply into attention-out.

Engine budget per core (approx): PE ~150us, ACT ~120us (exp + q/k PSUM
copies), DVE ~60us (v/y/accumulator copies + recips), GPSIMD ~60us
(causal-mask + normalize multiplies).
"""

import numpy as np

T = 2048
C = 768
G = 384          # channels per head-group (6 heads x 64)
DH = 64
NK = C // 128    # 6 k-tiles over c_in
TBLK = 512
NTB = T // TBLK  # 4 t-blocks
NST = T // 128   # 16 s-tiles
N_CORES = 8

_CACHE = {}


def _emit(tc, yT, xT, wq, wk, wv, wp, tri, dbg=None):
    import concourse.mybir as mybir
    import concourse.bass as bass

    nc = tc.nc
    # DRAM bounce buffer for the denominator-reciprocal broadcast: SBUF
    # sources cannot have a 0-step partition dim in DMA APs, DRAM can.
    # One slot per (t-block, head-pair): no reuse, no WAR.
    nrm_d = nc.dram_tensor("nrm_scratch", [NTB * 3, 2, TBLK],
                           mybir.dt.float32, kind="Internal").ap()
    nrm2_d = nc.dram_tensor("nrm2_scratch", [NTB * 3, 2, TBLK],
                            mybir.dt.float16, kind="Internal").ap()
    DT = mybir.dt.float32
    H = mybir.dt.float16
    Exp = mybir.ActivationFunctionType.Exp
    mm = nc.tensor.matmul

    with (
        tc.tile_pool(name="pc", bufs=1) as pc,        # persistent sbuf
        tc.tile_pool(name="px", bufs=2) as px,        # x chunks
        tc.tile_pool(name="pe", bufs=6) as pe,        # exp tiles
        tc.tile_pool(name="pr", bufs=3) as pr,        # y-out staging
        tc.tile_pool(name="pn", bufs=2) as pn,        # norm staging
        tc.tile_pool(name="pao", bufs=2) as pao,      # attn-out per t-block
        tc.tile_pool(name="psP", bufs=2, space="PSUM") as psP,    # scores
        tc.tile_pool(name="psQ", bufs=2, space="PSUM") as psQ,    # projections
        tc.tile_pool(name="psA", bufs=1, space="PSUM") as psA,    # attn accum head even
        tc.tile_pool(name="psB", bufs=1, space="PSUM") as psB,    # attn accum head odd
    ):
        # ---- persistent tensors ----
        wq_sb = pc.tile([128, NK * G], H, tag="wq")
        wk_sb = pc.tile([128, NK * G], H, tag="wk")
        wv_sb = pc.tile([128, NK * G], H, tag="wv")
        wp_sb = pc.tile([128, 3 * C], H, tag="wp")
        tri_sb = pc.tile([128, 128], H, tag="tri")

        # qT/kT: [128, 3*2048]; channel c of group -> partition c%128, block c//128.
        # head h (0..5): partitions (h%2)*64..+64 of block h//2.
        qT_sb = pc.tile([128, 3 * T], H, tag="qT")
        kT_sb = pc.tile([128, 3 * T], H, tag="kT")
        # v token-major with ones column: [128, 16, 6, 65];
        # v65[p, st, h, m] = v[st*128 + p, h*64 + m] for m < 64; col 64 = 1.0
        v65_sb = pc.tile([128, NST, 6, 65], H, tag="v65")

        xT_r = xT.rearrange("(k p) t -> p k t", p=128)
        wq_r = wq.rearrange("(k p) c -> p k c", p=128)
        wk_r = wk.rearrange("(k p) c -> p k c", p=128)
        wv_r = wv.rearrange("(k p) c -> p k c", p=128)
        wq_v = wq_sb.rearrange("p (k c) -> p k c", k=NK)
        wk_v = wk_sb.rearrange("p (k c) -> p k c", k=NK)
        wv_v = wv_sb.rearrange("p (k c) -> p k c", k=NK)

        def load_chunks(tb, split=False):
            xt = px.tile([128, NK, TBLK], H, tag="xc")
            if split:
                for k in range(NK):
                    nc.sync.dma_start(out=xt[:, k, :],
                                      in_=xT_r[:, k, tb * TBLK:(tb + 1) * TBLK])
            else:
                nc.sync.dma_start(out=xt[:], in_=xT_r[:, :, tb * TBLK:(tb + 1) * TBLK])
            return [xt[:, k, :] for k in range(NK)]

        # startup DMAs in priority order: (wq,x0) for the first q projection,
        # then wv (v groups interleave into tb=0 attention early), then wk,
        # then the late weights.
        for k in range(NK):
            nc.sync.dma_start(out=wq_v[:, k, :], in_=wq_r[:, k, :])
        xc_cur = load_chunks(0, split=True)
        for k in range(NK):
            nc.sync.dma_start(out=wk_v[:, k, :], in_=wk_r[:, k, :])
        for k in range(NK):
            nc.sync.dma_start(out=wv_v[:, k, :], in_=wv_r[:, k, :])
        nc.sync.dma_start(out=tri_sb[:], in_=tri)
        nc.sync.dma_start(out=wp_sb.rearrange("p (k c) -> p k c", k=3),
                          in_=wp.rearrange("(k p) c -> p k c", p=128))
        nc.gpsimd.memset(v65_sb[:, :, :, 64:65], 1.0)

        def ph1_groups(tb, xc):
            # closures: one projection matmul group each (q/k x 3, v x 4).
            # q/k PSUM->SBUF copies ride the ACT engine (DVE is the scarcer
            # resource); v copies stay on DVE (strided output).
            gs = []
            for w_sb, out_sb in ((wq_sb, qT_sb), (wk_sb, kT_sb)):
                for mo in range(3):
                    def g(w_sb=w_sb, out_sb=out_sb, mo=mo):
                        ps = psQ.tile([128, TBLK], DT, tag="pq")
                        for k in range(NK):
                            mm(ps[:, 0:TBLK],
                               lhsT=w_sb[:, k * G + mo * 128: k * G + (mo + 1) * 128],
                               rhs=xc[k], start=(k == 0), stop=(k == NK - 1))
                        nc.scalar.copy(
                            out=out_sb[:, mo * T + tb * TBLK: mo * T + (tb + 1) * TBLK],
                            in_=ps[:, 0:TBLK])
                    gs.append(g)
            for sl in range(4):
                def g(sl=sl):
                    st = 4 * tb + sl
                    ps = psQ.tile([128, TBLK], DT, tag="pq")
                    for k in range(NK):
                        mm(ps[:, 0:G], lhsT=xc[k][:, sl * 128:(sl + 1) * 128],
                           rhs=wv_sb[:, k * G:(k + 1) * G], start=(k == 0), stop=(k == NK - 1))
                    nc.vector.tensor_copy(
                        out=v65_sb[:, st, :, 0:64],
                        in_=ps[:, 0:G].rearrange("p (h m) -> p h m", h=6))
                gs.append(g)
            return gs

        def ph3_groups(tb, ao):
            gs = []
            for mo in range(6):
                def g(mo=mo):
                    py = psQ.tile([128, TBLK], DT, tag="pq")
                    for kk in range(3):
                        mm(py[:, 0:TBLK],
                           lhsT=wp_sb[:, kk * C + mo * 128: kk * C + (mo + 1) * 128],
                           rhs=ao[:, kk * TBLK:(kk + 1) * TBLK], start=(kk == 0), stop=(kk == 2))
                    yo = pr.tile([128, TBLK], H, tag="yo")
                    nc.vector.tensor_copy(out=yo[:], in_=py[:, 0:TBLK])
                    nc.sync.dma_start(
                        out=yT[mo * 128:(mo + 1) * 128, tb * TBLK:(tb + 1) * TBLK], in_=yo[:])
                gs.append(g)
            return gs

        # minimal serial head: just the mo=0 q/k projections; everything else
        # (v groups, mo=1/2 q/k) interleaves into tb=0's attention, ordered so
        # each group lands just before its first consumer.
        all0 = ph1_groups(0, xc_cur)    # [q0 q1 q2 k0 k1 k2 v0 v1 v2 v3]
        all0[0]()
        all0[3]()
        tb0_extra = [all0[1], all0[4], all0[6], all0[7],
                     all0[8], all0[2], all0[9], all0[5]]

        queue = []       # projection groups to interleave into phase 2
        ph3_pending = []
        for tb in range(NTB):
            if tb < NTB - 1:
                # ph1 first: those groups are dependency-free, so the PE queue
                # never head-of-line blocks on the previous block's norm chain
                xc_next = load_chunks(tb + 1)
                queue = ph1_groups(tb + 1, xc_next) + ph3_pending
            else:
                queue = list(ph3_pending)
            if tb == 0:
                queue = tb0_extra + queue
            total_iters = 3 * 4 * (tb + 1)
            emitted = [0]

            def make_pop(queue, emitted, total_iters):
                def pop_queue(it):
                    want = min(len(queue), (it + 1) * len(queue) // total_iters + 1)
                    while emitted[0] < want:
                        queue[emitted[0]]()
                        emitted[0] += 1
                return pop_queue
            pop_queue = make_pop(queue, emitted, total_iters)

            # ---- phase 2: attention for this t-block, head pairs j ----
            ao = pao.tile([128, 3 * TBLK], H, tag="ao")
            n_st = 4 * (tb + 1)

            norm_q = []

            def queue_norm(sA, sB, j_, tb_=tb):
                # softmax normalization, entirely off the PE path. The PSUM
                # accumulators were already freed by the per-region copies in
                # pv_group (sA/sB hold unnorm out rows 0:64 + den row 64).
                # recip (approx, DVE) -> DRAM bounce -> stride-0 DMA partition
                # broadcast -> GPSIMD multiplies into ao.
                sBp = pn.tile([128, TBLK], DT, tag="sBp")
                rr_in = pn.tile([64, 2, 8], DT, tag="rr_in")
                rr_out = pn.tile([64, 2, 8], H, tag="rr_out")
                dd = pn.tile([128, TBLK], H, tag="dd")
                slot = tb_ * 3 + j_

                nc.gpsimd.dma_start(out=sBp[64:128, :], in_=sB[0:64, :])

                # 1/den with a cheap DVE reciprocal: the 512-wide den row is
                # reshaped to [64 partitions x 8] through the DRAM bounce, so
                # the (free-size-proportional, ~6.4ns/elem) reciprocal costs
                # ~150ns instead of 3.3us and never backs up the DVE queue.
                def g4():
                    nc.gpsimd.dma_start(out=nrm_d[slot, 0, :], in_=sA[64:65, :])
                    nc.gpsimd.dma_start(out=nrm_d[slot, 1, :], in_=sB[64:65, :])

                def g5():
                    nc.gpsimd.dma_start(
                        out=rr_in[:],
                        in_=nrm_d[slot].rearrange("h (p e) -> p h e", p=64))

                def g6():
                    with nc.allow_low_precision(reason="1/den in fp16: 5e-4 rel"):
                        nc.vector.reciprocal(out=rr_out[:], in_=rr_in[:])

                def g7():
                    nc.gpsimd.dma_start(
                        out=nrm2_d[slot].rearrange("h (p e) -> p h e", p=64),
                        in_=rr_out[:])

                def g8():
                    for h, pp in ((0, slice(0, 64)), (1, slice(64, 128))):
                        row = nrm2_d[slot, h, :]
                        bc = bass.AP(tensor=row.tensor, offset=row.offset,
                                     ap=[[0, 64], [1, TBLK]])
                        nc.gpsimd.dma_start(out=dd[pp, :], in_=bc)

                def g9():
                    nc.gpsimd.tensor_mul(
                        ao[0:64, j_ * TBLK:(j_ + 1) * TBLK], sA[0:64, :], dd[0:64, :])

                def g10():
                    nc.gpsimd.tensor_mul(
                        ao[64:128, j_ * TBLK:(j_ + 1) * TBLK], sBp[64:128, :],
                        dd[64:128, :])

                norm_q.extend([g4, g5, g6, g7, g8, g9, g10])
                # (g4..g8 are DMA/DVE-cheap; g9/g10 are the GPSIMD multiplies)

            def pv_group(ent):
                # triangular PV: diagonal s-tiles only touch cols >= lo.
                # column region [128p, 128p+128) gets its final accumulation
                # at diagonal tile o == p -> stop=True there, and is copied
                # out to SBUF immediately (region-sized copies decouple the
                # next head-pair's PV start from one big end-of-j copy).
                j_, poA_, poB_, sA_, sB_, st, e01, first, lo = ent
                for h, po_, s_ in ((0, poA_, sA_), (1, poB_, sB_)):
                    v65 = v65_sb[:, st, 2 * j_ + h, :]
                    eh = e01[:, h * TBLK:(h + 1) * TBLK]
                    if lo < 0:       # off-diagonal: full width, never last
                        mm(po_[0:65, 0:TBLK], lhsT=v65, rhs=eh,
                           start=first, stop=False, skip_group_check=True)
                    else:
                        hi = lo + 128
                        mm(po_[0:65, lo:hi], lhsT=v65, rhs=eh[:, lo:hi],
                           start=first, stop=True, skip_group_check=True)
                        if hi < TBLK:
                            mm(po_[0:65, hi:TBLK], lhsT=v65, rhs=eh[:, hi:TBLK],
                               start=first, stop=False, skip_group_check=True)
                        nc.vector.tensor_copy(out=s_[0:65, lo:hi],
                                              in_=po_[0:65, lo:hi])

            pipe = []  # software pipeline: PV(st-2) is issued after scores(st)
            for j in range(3):
                poA = psA.tile([65, TBLK], DT, tag="poA")
                poB = psB.tile([65, TBLK], DT, tag="poB")
                sA = pn.tile([65, TBLK], DT, tag="sA")
                sB = pn.tile([65, TBLK], DT, tag="sB")
                qs = qT_sb[:, j * T + tb * TBLK: j * T + (tb + 1) * TBLK]
                for st in range(n_st):
                    o = st - 4 * tb           # >= 0 on diagonal tiles
                    lo = 128 * o if o >= 0 else -1
                    ks = kT_sb[:, j * T + st * 128: j * T + st * 128 + 128]
                    ps = psP.tile([128, 2 * TBLK], DT, tag="pp")
                    e01 = pe.tile([128, 2 * TBLK], H, tag="e01")
                    if o > 0:
                        # diagonal block: cols < lo are fully masked; compute
                        # scores/exp only on the remainder
                        mm(ps[:, lo:TBLK], lhsT=ks[0:64, :], rhs=qs[0:64, lo:TBLK],
                           start=True, stop=True)
                        mm(ps[:, TBLK + lo:2 * TBLK], lhsT=ks[64:128, :],
                           rhs=qs[64:128, lo:TBLK], start=True, stop=True)
                        nc.scalar.activation(
                            out=e01.rearrange("p (a b) -> p a b", a=2)[:, :, lo:],
                            in_=ps.rearrange("p (a b) -> p a b", a=2)[:, :, lo:],
                            func=Exp, scale=float(DH) ** -0.5)
                    else:
                        mm(ps[:, 0:TBLK], lhsT=ks[0:64, :], rhs=qs[0:64, :],
                           start=True, stop=True)
                        mm(ps[:, TBLK:2 * TBLK], lhsT=ks[64:128, :], rhs=qs[64:128, :],
                           start=True, stop=True)
                        nc.scalar.activation(out=e01[:], in_=ps[:], func=Exp,
                                             scale=float(DH) ** -0.5)
                    if o >= 0:  # triangular boundary block: causal mask (GPSIMD)
                        hi = lo + 128
                        nc.gpsimd.tensor_mul(e01[:, lo:hi], e01[:, lo:hi], tri_sb[:])
                        nc.gpsimd.tensor_mul(e01[:, TBLK + lo:TBLK + hi],
                                             e01[:, TBLK + lo:TBLK + hi], tri_sb[:])
                    pipe.append((j, poA, poB, sA, sB, st, e01, st == 0, lo))
                    if len(pipe) > 2:
                        pv_group(pipe.pop(0))
                    if norm_q:
                        norm_q.pop(0)()  # previous j's normalize, off the critical path
                    pop_queue(j * n_st + st)
                for ent in pipe:   # flush within j (poA/poB are single-buffered)
                    pv_group(ent)
                pipe = []
                queue_norm(sA, sB, j)
            while norm_q:
                norm_q.pop(0)()
            while emitted[0] < len(queue):
                queue[emitted[0]]()
                emitted[0] += 1
            ph3_pending = ph3_groups(tb, ao)
        for g in ph3_pending:  # tail: projection of the last t-block
            g()


def build_program():
    if "nc" in _CACHE:
        return _CACHE["nc"]
    import concourse.bacc as bacc
    import concourse.tile as tile
    import concourse.mybir as mybir

    nc = bacc.Bacc("TRN2", target_bir_lowering=False, debug=False)
    H = mybir.dt.float16
    xT_d = nc.dram_tensor("xT", [C, T], H, kind="ExternalInput")
    wq_d = nc.dram_tensor("wq", [C, G], H, kind="ExternalInput")
    wk_d = nc.dram_tensor("wk", [C, G], H, kind="ExternalInput")
    wv_d = nc.dram_tensor("wv", [C, G], H, kind="ExternalInput")
    wp_d = nc.dram_tensor("wp", [G, C], H, kind="ExternalInput")
    tri_d = nc.dram_tensor("tri", [128, 128], H, kind="ExternalInput")
    yT_d = nc.dram_tensor("yT", [C, T], H, kind="ExternalOutput")

    with tile.TileContext(nc) as tc:
        _emit(tc, yT_d.ap(), xT_d.ap(), wq_d.ap(), wk_d.ap(), wv_d.ap(),
              wp_d.ap(), tri_d.ap())
    nc.compile()
    _CACHE["nc"] = nc
    return nc


def make_tri():
    s = np.arange(128)[:, None]
    t = np.arange(128)[None, :]
    return (t >= s).astype(np.float16)


def shard_inputs(x, Wq, Wk, Wv, Wp):
    """Full inputs -> list of 8 per-core input dicts (fp16 operands)."""
    x = np.asarray(x, dtype=np.float32)
    Wq, Wk, Wv, Wp = (np.asarray(w, dtype=np.float32) for w in (Wq, Wk, Wv, Wp))
    tri = make_tri()
    in_maps = []
    for c in range(N_CORES):
        b, g = divmod(c, 2)
        sl = slice(g * G, (g + 1) * G)
        in_maps.append({
            "xT": np.ascontiguousarray(x[b].T).astype(np.float16),
            "wq": np.ascontiguousarray(Wq[sl, :].T).astype(np.float16),
            "wk": np.ascontiguousarray(Wk[sl, :].T).astype(np.float16),
            "wv": np.ascontiguousarray(Wv[sl, :].T).astype(np.float16),
            "wp": np.ascontiguousarray(Wp[:, sl].T).astype(np.float16),
            "tri": tri,
        })
    return in_maps


def combine_outputs(results):
    """Per-core {'yT': [768,2048]} partials -> full [4, 2048, 768] output."""
    out = np.empty((4, T, C), dtype=np.float32)
    for b in range(4):
        acc = (results[2 * b]["yT"].astype(np.float32)
               + results[2 * b + 1]["yT"].astype(np.float32))
        out[b] = acc.T
    return out


def kernel(x, Wq, Wk, Wv, Wp, **run_kwargs):
    from concourse.bass_utils import run_bass_kernel_spmd

    nc = build_program()
    in_maps = shard_inputs(x, Wq, Wk, Wv, Wp)
    res = run_bass_kernel_spmd(nc, in_maps, core_ids=list(range(N_CORES)), **run_kwargs)
    out = combine_outputs(res.results)
    if run_kwargs:
        return out, res
    return out
